# revision 27
# baseline (speedup 1.0000x reference)
"""ExpertRouter (MoE gating) Trainium2 Bass kernel.

Math (matches the fp32 jax reference):
    h      = silu(x @ W1 + b1)          [B, HID]
    logits = h @ W2 + b2                [B, E]
    top_vals, top_idx = top_k(logits, 2)
    weights  = softmax(top_vals)        [B, 2]
    probs    = softmax(logits)          [B, E]
    freq     = mean(one_hot(argmax(logits)))   # [E]
    aux_loss = E * sum(freq * mean(probs, 0))

Sharding: data-parallel over the token dim B across 8 NeuronCores.  Gate
weights are replicated; each core returns its weights/top_idx shard plus
per-expert partial column sums (one-hot counts and prob sums) which the
host combines into aux_loss.

Per-core pipeline (tokens chunked by TCHUNK):
  A. load x tiles (natural layout), transpose 128x128 blocks on the PE
     (contraction must sit on the partition dim) into XT [128, D/128, TCHUNK]
  B. H.T tile [128j, TCHUNK] = sum_d W1[d,j].T @ XT[d,:]  (PSUM accum),
     then silu(h + b1) with b1 as a per-partition ACT bias (sigmoid+mul so
     the kernel also runs under CoreSim, which lacks a Silu table)
  C. logits [128t, E] = sum_j H.T[:, t].T @ W2[j,:]  (+ b2 via a K=1
     ones-row matmul)
  D. Max8 -> top-2 vals/idx, softmax weights, probs, one-hot(argmax);
     per-expert column sums via [128,E].T @ ones accumulated in PSUM
     across the whole kernel.

The matmul input dtype for the big GEMM is switchable between float32
(4 cyc/row) and float32r (1 cyc/row at N>=256).
"""

import os
from contextlib import ExitStack

import numpy as np

import concourse.bass as bass
import concourse.mybir as mybir
import concourse.tile as tile

AF = mybir.ActivationFunctionType
ALU = mybir.AluOpType
DT = mybir.dt

N_CORES = 8
B_FULL, D_FULL, HID_FULL, E_FULL = 16384, 4096, 2048, 64

FULL_CFG = dict(
    b=B_FULL // N_CORES,  # tokens per core
    d=D_FULL,
    hid=HID_FULL,
    e=E_FULL,
    tchunk=512,
    mm_dtype=os.environ.get("ROUTER_MM_DTYPE", "float32"),
)


def build_kernel(nc: bass.Bass, cfg: dict) -> None:
    b, d, hid, e = cfg["b"], cfg["d"], cfg["hid"], cfg["e"]
    tchunk = cfg["tchunk"]
    mmdt = None if cfg["mm_dtype"] == "fp16x3" else getattr(DT, cfg["mm_dtype"])

    nchunk = b // tchunk  # token chunks per core
    t128 = tchunk // 128  # 128-token blocks per chunk
    nd = d // 128  # contraction blocks for matmul1
    nh = d // 512  # 512-wide column groups of x
    nj = hid // 128  # HID blocks
    assert b % tchunk == 0 and tchunk % 128 == 0 and d % 512 == 0
    assert hid % 128 == 0 and tchunk <= 512 and 8 <= e <= 128
    fp16x3 = cfg["mm_dtype"] == "fp16x3"
    W1_SCALE = 1024.0  # keeps fp16 lo-part of W1 (~0.02 scale) out of subnormals

    x = nc.dram_tensor("x", [b, d], DT.float32, kind="ExternalInput").ap()
    w1 = nc.dram_tensor("W1", [d, hid], DT.float32, kind="ExternalInput").ap()
    b1 = nc.dram_tensor("b1", [hid], DT.float32, kind="ExternalInput").ap()
    w2 = nc.dram_tensor("W2", [hid, e], DT.float32, kind="ExternalInput").ap()
    b2 = nc.dram_tensor("b2", [e], DT.float32, kind="ExternalInput").ap()

    ow = nc.dram_tensor("out_w", [b, 2], DT.float32, kind="ExternalOutput").ap()
    oi = nc.dram_tensor("out_i", [b, 2], DT.int32, kind="ExternalOutput").ap()
    ofs = nc.dram_tensor("out_fs", [e, 1], DT.float32, kind="ExternalOutput").ap()
    ops = nc.dram_tensor("out_ps", [e, 1], DT.float32, kind="ExternalOutput").ap()

    with tile.TileContext(nc) as tc, ExitStack() as ctx:
        const = ctx.enter_context(tc.tile_pool(name="const", bufs=1))
        xn_pool = ctx.enter_context(tc.tile_pool(name="xn", bufs=8))
        xt_pool = ctx.enter_context(tc.tile_pool(name="xt", bufs=1))
        w1_pool = ctx.enter_context(tc.tile_pool(name="w1c", bufs=3))
        ht_pool = ctx.enter_context(tc.tile_pool(name="ht", bufs=1))
        sm_pool = ctx.enter_context(tc.tile_pool(name="sm", bufs=3))
        out_pool = ctx.enter_context(tc.tile_pool(name="outp", bufs=3))
        tpsum = ctx.enter_context(tc.tile_pool(name="tps", bufs=2, space="PSUM"))
        hpsum = ctx.enter_context(tc.tile_pool(name="hps", bufs=1, space="PSUM"))
        lpsum = ctx.enter_context(tc.tile_pool(name="lps", bufs=1, space="PSUM"))
        apsum = ctx.enter_context(tc.tile_pool(name="aps", bufs=1, space="PSUM"))

        # ---- constants -------------------------------------------------
        iota_ri = const.tile([128, 128], DT.int32, name="iota_ri")
        nc.gpsimd.iota(iota_ri, pattern=[[1, 128]], base=0, channel_multiplier=0)
        iota_rf = const.tile([128, 128], DT.float32, name="iota_rf")
        nc.vector.tensor_copy(iota_rf, iota_ri)
        pcol_i = const.tile([128, 1], DT.int32, name="pcol_i")
        nc.gpsimd.iota(pcol_i, pattern=[[1, 1]], base=0, channel_multiplier=1)
        pcol_f = const.tile([128, 1], DT.float32, name="pcol_f")
        nc.vector.tensor_copy(pcol_f, pcol_i)
        ident = const.tile([128, 128], DT.float32, name="ident")
        nc.vector.tensor_scalar(ident, iota_rf, pcol_f, None, op0=ALU.is_equal)
        iota_f = const.tile([128, e], DT.float32, name="iota_f")
        nc.vector.tensor_copy(iota_f, iota_ri[:, 0:e])

        ones_col = const.tile([128, 1], DT.float32, name="ones_col")
        nc.vector.memset(ones_col, 1.0)
        ones_row = const.tile([1, 128], DT.float32, name="ones_row")
        nc.vector.memset(ones_row, 1.0)

        b2t = const.tile([1, e], DT.float32, name="b2t")
        nc.sync.dma_start(b2t, b2.rearrange("(a e) -> a e", a=1))
        # b2 replicated across partitions via a K=1 PE outer product
        ps_b2 = tpsum.tile([128, e], DT.float32, name="ps_b2", tag="ps_t")
        nc.tensor.matmul(ps_b2, lhsT=ones_row, rhs=b2t, start=True, stop=True)
        b2full = const.tile([128, e], DT.float32, name="b2full")
        nc.vector.tensor_copy(b2full, ps_b2)
        b1n = const.tile([nj, 128], DT.float32, name="b1n")
        nc.sync.dma_start(b1n, b1.rearrange("(a p) -> a p", p=128))
        ps_b1 = tpsum.tile([128, nj], DT.float32, name="ps_b1", tag="ps_t")
        nc.tensor.transpose(ps_b1, b1n, ident[:nj, :nj])
        b1t = const.tile([128, nj], DT.float32, name="b1t")
        nc.vector.tensor_copy(b1t, ps_b1)

        w2t = const.tile([128, nj, e], DT.float32, name="w2t")
        nc.sync.dma_start(w2t, w2.rearrange("(a p) j -> p a j", p=128))

        # whole-kernel PSUM accumulator for the per-expert column sums:
        # partitions 0:e = one-hot counts, partitions e:2e = prob sums
        acc2 = apsum.tile([2 * e, 1], DT.float32, name="acc2", tag="acc2")

        if fp16x3:
            # prepass: W1*S split into fp16 hi/lo, staged in DRAM scratch
            # (natural-mirror layout [nd, 128, hid]); column-block outer so
            # stage B of chunk 0 can start after the first block.
            dram_pool = ctx.enter_context(tc.tile_pool(name="w1dram", bufs=1, space="DRAM"))
            w1sp = ctx.enter_context(tc.tile_pool(name="w1sp", bufs=3))
            w1h_dram = dram_pool.tile([nd, 128, hid], DT.float16, name="w1h_dram", tag="w1h_dram")
            w1l_dram = dram_pool.tile([nd, 128, hid], DT.float16, name="w1l_dram", tag="w1l_dram")
            for colblk in range(hid // 512):
                for k in range(nd):
                    wn = xn_pool.tile([128, 512], DT.float32, name="wn", tag="xn")
                    nc.sync.dma_start(
                        wn, w1[k * 128 : (k + 1) * 128, colblk * 512 : (colblk + 1) * 512]
                    )
                    wh = w1sp.tile([128, 512], DT.float16, name="wh", tag="wh")
                    nc.scalar.activation(wh, wn, AF.Copy, scale=W1_SCALE)
                    wbk = w1sp.tile([128, 512], DT.float32, name="wbk", tag="wbk")
                    nc.vector.tensor_copy(wbk, wh)
                    wl = w1sp.tile([128, 512], DT.float16, name="wl", tag="wl")
                    nc.vector.scalar_tensor_tensor(
                        wl, wn, W1_SCALE, wbk, op0=ALU.mult, op1=ALU.subtract
                    )
                    nc.sync.dma_start(
                        w1h_dram[k, :, colblk * 512 : (colblk + 1) * 512], wh
                    )
                    nc.sync.dma_start(
                        w1l_dram[k, :, colblk * 512 : (colblk + 1) * 512], wl
                    )

        for rep in range(cfg.get("reps", 1)):
          for c in range(nchunk):
            # ---- stage A: x -> XT (transpose via PE) -------------------
            if fp16x3:
                xh_big = xt_pool.tile([128, nd, tchunk], DT.float16, name="xh_big", tag="xh")
                xl_big = xt_pool.tile([128, nd, tchunk], DT.float16, name="xl_big", tag="xl")
            else:
                xt_big = xt_pool.tile([128, nd, tchunk], DT.float32, name="xt_big", tag="xt")
            for h in range(nh):
                xnats = []
                for td in range(t128):
                    xn = xn_pool.tile([128, 512], DT.float32, name="xn", tag="xn")
                    r0 = c * tchunk + td * 128
                    nc.sync.dma_start(xn, x[r0 : r0 + 128, h * 512 : (h + 1) * 512])
                    xnats.append(xn)
                for sub in range(4):
                    dd = h * 4 + sub
                    pst = tpsum.tile([128, tchunk], DT.float32, name="pst", tag="ps_t")
                    for td in range(t128):
                        nc.tensor.transpose(
                            pst[:, td * 128 : (td + 1) * 128],
                            xnats[td][:, sub * 128 : (sub + 1) * 128],
                            ident,
                        )
                    if fp16x3:
                        nc.vector.tensor_copy(xh_big[:, dd, :], pst)
                        xbk = sm_pool.tile([128, tchunk], DT.float32, name="xbk", tag="xbk")
                        nc.vector.tensor_copy(xbk, xh_big[:, dd, :])
                        nc.vector.scalar_tensor_tensor(
                            xl_big[:, dd, :], pst, 1.0, xbk, op0=ALU.mult, op1=ALU.subtract
                        )
                    else:
                        nc.vector.tensor_copy(xt_big[:, dd, :], pst)

            # ---- stage B: H.T tiles = silu(W1.T @ X.T + b1) ------------
            if not fp16x3:
                # stream W1 as full-rate row tiles [128d, 512j]; accumulate a
                # 4-wide jj group in 4 PSUM banks so each W1 tile is consumed
                # by 4 matmuls right after its (contiguous) DMA lands.
                hts = [None] * nj
                for grp in range(nj // 4):
                    phs = [
                        hpsum.tile([128, tchunk], DT.float32, name="ph", tag=f"ph{q}")
                        for q in range(4)
                    ]
                    for dd in range(nd):
                        w1t = w1_pool.tile([128, 512], DT.float32, name="w1t", tag="w1t", bufs=6)
                        nc.sync.dma_start(
                            w1t,
                            w1[dd * 128 : (dd + 1) * 128, grp * 512 : (grp + 1) * 512],
                        )
                        for q in range(4):
                            nc.tensor.matmul(
                                phs[q],
                                lhsT=w1t[:, q * 128 : (q + 1) * 128].bitcast(mmdt),
                                rhs=xt_big[:, dd, :].bitcast(mmdt),
                                start=(dd == 0),
                                stop=(dd == nd - 1),
                            )
                    for q in range(4):
                        jj = grp * 4 + q
                        hb = ht_pool.tile([128, tchunk], DT.float32, name="hb", tag=f"hb{jj}")
                        nc.scalar.activation(
                            hb, phs[q], AF.Identity, bias=b1t[:, jj : jj + 1], scale=1.0
                        )
                        sg = sm_pool.tile([128, tchunk], DT.float32, name="sg", tag="sg")
                        nc.scalar.activation(sg, hb, AF.Sigmoid)
                        nc.vector.tensor_mul(hb, hb, sg)
                        hts[jj] = hb
            else:
              hts = []
              for jj in range(nj):
                ph = hpsum.tile([128, tchunk], DT.float32, name="ph", tag="ph")
                if fp16x3:
                    w1h_sb = w1_pool.tile([128, nd, 128], DT.float16, name="w1h_sb", tag="w1h", bufs=2)
                    nc.sync.dma_start(
                        w1h_sb,
                        w1h_dram[:, :, jj * 128 : (jj + 1) * 128].rearrange("k p j -> p k j"),
                    )
                    w1l_sb = w1_pool.tile([128, nd, 128], DT.float16, name="w1l_sb", tag="w1l", bufs=2)
                    nc.sync.dma_start(
                        w1l_sb,
                        w1l_dram[:, :, jj * 128 : (jj + 1) * 128].rearrange("k p j -> p k j"),
                    )
                    for dd in range(nd):
                        nc.tensor.matmul(
                            ph, lhsT=w1h_sb[:, dd, :], rhs=xh_big[:, dd, :],
                            start=(dd == 0), stop=False,
                        )
                        nc.tensor.matmul(
                            ph, lhsT=w1h_sb[:, dd, :], rhs=xl_big[:, dd, :],
                            start=False, stop=False,
                        )
                        nc.tensor.matmul(
                            ph, lhsT=w1l_sb[:, dd, :], rhs=xh_big[:, dd, :],
                            start=False, stop=(dd == nd - 1),
                        )
                else:
                    w1c = w1_pool.tile([128, nd, 128], DT.float32, name="w1c", tag="w1c")
                    nc.sync.dma_start(
                        w1c, w1[:, jj * 128 : (jj + 1) * 128].rearrange("(k p) j -> p k j", p=128)
                    )
                    for dd in range(nd):
                        nc.tensor.matmul(
                            ph,
                            lhsT=w1c[:, dd, :].bitcast(mmdt),
                            rhs=xt_big[:, dd, :].bitcast(mmdt),
                            start=(dd == 0),
                            stop=(dd == nd - 1),
                        )
                hb = ht_pool.tile([128, tchunk], DT.float32, name="hb", tag=f"hb{jj}")
                nc.scalar.activation(
                    hb, ph, AF.Identity, bias=b1t[:, jj : jj + 1],
                    scale=(1.0 / W1_SCALE) if fp16x3 else 1.0,
                )
                sg = sm_pool.tile([128, tchunk], DT.float32, name="sg", tag="sg")
                nc.scalar.activation(sg, hb, AF.Sigmoid)
                nc.vector.tensor_mul(hb, hb, sg)
                hts.append(hb)

            # ---- stage C+D: logits, top-2, softmax, partial sums -------
            for td in range(t128):
                r0 = c * tchunk + td * 128
                pl = lpsum.tile([128, e], DT.float32, name="pl", tag="pl")
                for jj in range(nj):
                    nc.tensor.matmul(
                        pl,
                        lhsT=hts[jj][:, td * 128 : (td + 1) * 128],
                        rhs=w2t[:, jj, :],
                        start=(jj == 0),
                        stop=(jj == nj - 1),
                    )

                # logits = pl + b2 (fused into the PSUM->SBUF move)
                lg = sm_pool.tile([128, e], DT.float32, name="lg", tag="lg")
                nc.vector.scalar_tensor_tensor(
                    lg, pl, 1.0, b2full, op0=ALU.mult, op1=ALU.add
                )
                mx8 = sm_pool.tile([128, 8], DT.float32, name="mx8", tag="mx8")
                nc.vector.max(mx8, lg)
                ix8 = sm_pool.tile([128, 8], DT.uint32, name="ix8", tag="ix8")
                nc.vector.max_index(ix8, mx8, lg)

                # weights = softmax([v1, v2]) = [1/(1+t), t/(1+t)], t=exp(v2-v1)
                dlt = sm_pool.tile([128, 1], DT.float32, name="dlt", tag="dlt")
                nc.vector.tensor_sub(dlt, mx8[:, 1:2], mx8[:, 0:1])
                ev = sm_pool.tile([128, 1], DT.float32, name="ev", tag="ev")
                nc.scalar.activation(ev, dlt, AF.Exp)
                den = sm_pool.tile([128, 1], DT.float32, name="den", tag="den")
                nc.vector.tensor_scalar_add(den, ev, 1.0)
                rr = sm_pool.tile([128, 1], DT.float32, name="rr", tag="rr")
                nc.vector.reciprocal(rr, den)
                outw = out_pool.tile([128, 2], DT.float32, name="outw", tag="outw")
                nc.vector.tensor_copy(outw[:, 0:1], rr)
                nc.vector.tensor_mul(outw[:, 1:2], ev, rr)
                nc.sync.dma_start(ow[r0 : r0 + 128, :], outw)

                outi = out_pool.tile([128, 2], DT.int32, name="outi", tag="outi")
                nc.vector.tensor_copy(outi, ix8[:, 0:2])
                nc.sync.dma_start(oi[r0 : r0 + 128, :], outi)

                # probs = exp(lg - v1) / rowsum
                negv = sm_pool.tile([128, 1], DT.float32, name="negv", tag="negv")
                nc.vector.tensor_scalar_mul(negv, mx8[:, 0:1], -1.0)
                pr = sm_pool.tile([128, e], DT.float32, name="pr", tag="pr")
                sume = sm_pool.tile([128, 1], DT.float32, name="sume", tag="sume")
                nc.scalar.activation(pr, lg, AF.Exp, bias=negv, scale=1.0, accum_out=sume)
                rs = sm_pool.tile([128, 1], DT.float32, name="rs", tag="rs")
                nc.vector.reciprocal(rs, sume)

                # mq = [one_hot(argmax) | normalized probs]; one matmul
                # accumulates both per-expert column sums into acc2
                mq = sm_pool.tile([128, 2 * e], DT.float32, name="mq", tag="mq")
                nc.vector.tensor_scalar_mul(mq[:, e : 2 * e], pr, rs)
                ixf = sm_pool.tile([128, 1], DT.float32, name="ixf", tag="ixf")
                nc.vector.tensor_copy(ixf, ix8[:, 0:1])
                nc.vector.tensor_scalar(mq[:, 0:e], iota_f, ixf, None, op0=ALU.is_equal)

                first = c == 0 and td == 0
                last = c == nchunk - 1 and td == t128 - 1
                nc.tensor.matmul(acc2, lhsT=mq, rhs=ones_col, start=first, stop=last)

        accs = out_pool.tile([2 * e, 1], DT.float32, name="accs", tag="accs")
        nc.vector.tensor_copy(accs, acc2)
        nc.sync.dma_start(ofs, accs[0:e, :])
        nc.sync.dma_start(ops, accs[e : 2 * e, :])


def build_nc(cfg: dict) -> bass.Bass:
    import concourse.bacc as bacc

    nc = bacc.Bacc(trn_type="TRN2", debug=False)
    build_kernel(nc, cfg)
    nc.compile()  # splits multi-wait instructions (HW allows 1 wait/inst)
    return nc


_NC_CACHE: dict = {}


def _get_nc() -> bass.Bass:
    key = FULL_CFG["mm_dtype"]
    if key not in _NC_CACHE:
        _NC_CACHE[key] = build_nc(FULL_CFG)
    return _NC_CACHE[key]


def _combine(results: list, n_tokens: int, e: int):
    weights = np.concatenate([r["out_w"] for r in results], axis=0)
    top_idx = np.concatenate([r["out_i"] for r in results], axis=0)
    fsum = np.sum([r["out_fs"][:, 0].astype(np.float64) for r in results], axis=0)
    psum = np.sum([r["out_ps"][:, 0].astype(np.float64) for r in results], axis=0)
    freq = fsum / n_tokens
    avg_prob = psum / n_tokens
    aux = np.float32(e * np.sum(freq * avg_prob))
    return weights.astype(np.float32), top_idx.astype(np.int32), aux


def _make_runner(nc):
    """Jit-once PJRT runner over the 8 axon cores.

    Same execution path run_bass_kernel_spmd takes under axon
    (bass2jax/_bass_exec_p -> neuronx_cc_hook -> NEFF via PJRT), but the
    jitted callable is cached so repeat kernel() calls don't re-trace, and
    the replicated gate weights are passed via shard_map in_specs instead
    of being concatenated 8x.
    """
    import jax
    from jax.sharding import Mesh, PartitionSpec

    from jax.experimental.shard_map import shard_map

    import concourse.mybir as mybir
    from concourse import bass2jax

    bass2jax.install_neuronx_cc_hook()
    assert nc.dbg_addr is None
    partition_name = nc.partition_id_tensor.name if nc.partition_id_tensor else None

    sharded_inputs = {"x"}
    in_names, out_names, out_avals, zero_shapes = [], [], [], []
    for alloc in nc.m.functions[0].allocations:
        if not isinstance(alloc, mybir.MemoryLocationSet):
            continue
        name = alloc.memorylocations[0].name
        if alloc.kind == "ExternalInput":
            if name != partition_name:
                in_names.append(name)
        elif alloc.kind == "ExternalOutput":
            out_names.append(name)
            shape = tuple(alloc.tensor_shape)
            dtype = mybir.dt.np(alloc.dtype)
            out_avals.append(jax.core.ShapedArray(shape, dtype))
            zero_shapes.append((shape, dtype))
    n_params = len(in_names)
    n_outs = len(out_avals)
    all_in_names = list(in_names) + list(out_names)
    if partition_name is not None:
        all_in_names.append(partition_name)
    donate = tuple(range(n_params, n_params + n_outs))

    def _body(*args):
        operands = list(args)
        if partition_name is not None:
            operands.append(bass2jax.partition_id_tensor())
        outs = bass2jax._bass_exec_p.bind(
            *operands,
            out_avals=tuple(out_avals),
            in_names=tuple(all_in_names),
            out_names=tuple(out_names),
            lowering_input_output_aliases=(),
            sim_require_finite=True,
            sim_require_nnan=True,
            nc=nc,
        )
        return tuple(outs)

    devices = jax.devices()[:N_CORES]
    mesh = Mesh(np.asarray(devices), ("core",))
    in_specs = tuple(
        PartitionSpec("core") if name in sharded_inputs else PartitionSpec()
        for name in in_names
    ) + (PartitionSpec("core"),) * n_outs
    out_specs = (PartitionSpec("core"),) * n_outs
    sharded = jax.jit(
        shard_map(_body, mesh=mesh, in_specs=in_specs, out_specs=out_specs, check_rep=False),
        donate_argnums=donate,
        keep_unused=True,
    )

    def run(inputs: dict):
        args = [inputs[name] for name in in_names]
        zeros = [np.zeros((N_CORES * s[0], *s[1:]), dt) for s, dt in zero_shapes]
        out_arrs = sharded(*args, *zeros)
        return [
            {
                name: np.asarray(out_arrs[i]).reshape(N_CORES, *out_avals[i].shape)[c]
                for i, name in enumerate(out_names)
            }
            for c in range(N_CORES)
        ]

    return run


_RUNNER_CACHE: dict = {}


def kernel(**inputs) -> tuple:
    x = np.ascontiguousarray(np.asarray(inputs["x"], dtype=np.float32))
    full = {
        "x": x,
        "W1": np.ascontiguousarray(np.asarray(inputs["W1"], dtype=np.float32)),
        "b1": np.ascontiguousarray(np.asarray(inputs["b1"], dtype=np.float32)),
        "W2": np.ascontiguousarray(np.asarray(inputs["W2"], dtype=np.float32)),
        "b2": np.ascontiguousarray(np.asarray(inputs["b2"], dtype=np.float32)),
    }
    key = FULL_CFG["mm_dtype"]
    try:
        if key not in _RUNNER_CACHE:
            _RUNNER_CACHE[key] = _make_runner(_get_nc())
        results = _RUNNER_CACHE[key](full)
    except Exception:
        # fallback: the stock SPMD runner (same execution path, uncached)
        from concourse.bass_utils import run_bass_kernel_spmd

        b_core = x.shape[0] // N_CORES
        in_maps = [
            {**{k: v for k, v in full.items() if k != "x"},
             "x": full["x"][i * b_core : (i + 1) * b_core]}
            for i in range(N_CORES)
        ]
        results = run_bass_kernel_spmd(
            _get_nc(), in_maps, core_ids=list(range(N_CORES))
        ).results
    return _combine(results, x.shape[0], FULL_CFG["e"])
